# revision 12
# baseline (speedup 1.0000x reference)
"""MARN (memory-augmented RNN) cell — Trainium2 Bass kernel, 8-core data parallel.

Problem: nn_MARN_73065983640197. B=4096 batch sharded 512/core; weights replicated.
Outputs (h_curr, c, M_curr, k_curr) matching the jax reference.

Per-core dataflow (batch-major tiles of 128 rows):
  gates/sigmoid/tanh on PE+ACT (u-major via PE transposes),
  cosine-similarity softmax over the (64,256) memory via DVE fused reduce ops,
  memory update M' = M*(1-a*e) + a*a via 2 fused DVE passes in-place,
  row norms on ACT (Square+accum), rsqrt via fp32 bit-trick + Newton (DVE).
"""
import sys
import numpy as np

for _p in ("/opt/trn_rl_repo", "/root/.axon_site/_ro/trn_rl_repo"):
    if _p not in sys.path:
        sys.path.insert(0, _p)

import concourse.bacc as bacc
import concourse.tile as tile
from concourse import mybir
from concourse.bass_utils import run_bass_kernel_spmd
from concourse.masks import make_identity

N_CORES = 8
B, IN_DIM, U, NK, S = 4096, 128, 256, 64, 256
BC = B // N_CORES          # 512 rows per core
P = 128                    # partition tile
NT = BC // P               # 4 batch tiles per core
F32 = mybir.dt.float32
I32 = mybir.dt.int32
ALU = mybir.AluOpType
ACTF = mybir.ActivationFunctionType
AX = mybir.AxisListType

DEBUG_TAPS = False
WEIGHT_NAMES = ["Wf", "Wt", "Wi", "Wo", "Wk", "Wr", "Wc", "Wh", "We", "Wa"]
BIAS_NAMES = ["bf", "bt", "bi", "bo", "bk", "be", "ba"]


def _emit_rsqrt(nc, pool, x_ap, n, tag, iters=2):
    """y = 1/sqrt(x) for x>0, fp32 bit-trick seed + NR. x_ap: (P, n) f32 AP."""
    y = pool.tile([P, n], F32, tag=f"rsq_y_{tag}")
    t1 = pool.tile([P, n], F32, tag=f"rsq_t_{tag}")
    yi = y[:].bitcast(I32)
    # C - (bits>>1) == ((bits>>1) ^ -1) + (C+1); walrus can't mix bitwise+arith
    nc.vector.tensor_scalar(yi, x_ap.bitcast(I32), 1, None, ALU.arith_shift_right)
    nc.vector.tensor_scalar(yi, yi, -1, None, ALU.bitwise_xor)
    nc.vector.tensor_scalar(yi, yi, 0x5F3759E0, None, ALU.add)
    for _ in range(iters):
        nc.vector.tensor_tensor(t1[:], y[:], y[:], ALU.mult)
        nc.vector.tensor_tensor(t1[:], t1[:], x_ap, ALU.mult)
        nc.vector.tensor_scalar(t1[:], t1[:], -0.5, 1.5, ALU.mult, ALU.add)
        nc.vector.tensor_tensor(y[:], y[:], t1[:], ALU.mult)
    return y


def emit_core(ctx, tc):
    nc = tc.nc
    dram = {}
    dram["X"] = nc.dram_tensor("X", [BC, IN_DIM], F32, kind="ExternalInput").ap()
    dram["h_prev"] = nc.dram_tensor("h_prev", [BC, U], F32, kind="ExternalInput").ap()
    dram["c_prev"] = nc.dram_tensor("c_prev", [BC, U], F32, kind="ExternalInput").ap()
    dram["M_prev"] = nc.dram_tensor("M_prev", [BC, NK, S], F32, kind="ExternalInput").ap()
    dram["k_prev"] = nc.dram_tensor("k_prev", [BC, S], F32, kind="ExternalInput").ap()
    for w in WEIGHT_NAMES:
        rows = IN_DIM + U if w in ("Wf", "Wt", "Wi", "Wo") else U if w not in ("Wr", "Wh") else S
        dram[w] = nc.dram_tensor(w, [rows, U if w not in ("Wk", "We", "Wa") else S], F32,
                                 kind="ExternalInput").ap()
    for b in BIAS_NAMES:
        dram[b] = nc.dram_tensor(b, [S if b in ("bk", "be", "ba") else U], F32,
                                 kind="ExternalInput").ap()
    dram["h_out"] = nc.dram_tensor("h_out", [BC, U], F32, kind="ExternalOutput").ap()
    dram["c_out"] = nc.dram_tensor("c_out", [BC, U], F32, kind="ExternalOutput").ap()
    dram["M_out"] = nc.dram_tensor("M_out", [BC, NK, S], F32, kind="ExternalOutput").ap()
    dram["k_out"] = nc.dram_tensor("k_out", [BC, S], F32, kind="ExternalOutput").ap()
    if DEBUG_TAPS:
        for nm, shp in [("dbg_num", [P, NK]), ("dbg_nsq", [P, NK]), ("dbg_cos", [P, NK]),
                        ("dbg_alpha", [P, NK]), ("dbg_r", [P, S]), ("dbg_eb", [P, S]),
                        ("dbg_ab", [P, S]), ("dbg_kn", [P, S]), ("dbg_rc1", [P, 2, P]),
                        ("dbg_hT", [P, 2, P])]:
            dram[nm] = nc.dram_tensor(nm, shp, F32, kind="ExternalOutput").ap()

    const = ctx.enter_context(tc.tile_pool(name="const", bufs=1))
    mpool = ctx.enter_context(tc.tile_pool(name="mpool", bufs=3))
    work = ctx.enter_context(tc.tile_pool(name="work", bufs=2))
    scr = ctx.enter_context(tc.tile_pool(name="scr", bufs=2))
    tp_psum = ctx.enter_context(tc.tile_pool(name="tp_psum", bufs=3, space="PSUM"))
    mm_psum = ctx.enter_context(tc.tile_pool(name="mm_psum", bufs=4, space="PSUM"))

    # ---- constants: identity + weights (f-major chunks) + biases ----
    ident = const.tile([P, P], F32)
    make_identity(nc, ident[:])

    wsb = {}
    for w in ["Wf", "Wt", "Wi", "Wo"]:
        t = const.tile([P, 3, U], F32, tag=f"w_{w}")
        nc.sync.dma_start(t[:], dram[w].rearrange("(c p) u -> p c u", p=P))
        wsb[w] = t
    for w in ["Wk", "Wr", "Wc", "Wh", "We", "Wa"]:
        t = const.tile([P, 2, S], F32, tag=f"w_{w}")
        nc.sync.dma_start(t[:], dram[w].rearrange("(c p) u -> p c u", p=P))
        wsb[w] = t
    bsb = {}
    for b in BIAS_NAMES:
        t = const.tile([P, 2], F32, tag=f"b_{b}")
        nc.sync.dma_start(t[:], dram[b].rearrange("(c p) -> p c", p=P))
        bsb[b] = t
    # halved copies for sigmoid-via-tanh: tanh(0.5 z + 0.5 b)
    bhalf = {}
    for b in ["bf", "bt", "bi", "bo", "be"]:
        t = const.tile([P, 2], F32, tag=f"bh_{b}")
        nc.vector.tensor_scalar_mul(t[:], bsb[b][:], 0.5)
        bhalf[b] = t

    def transpose128(dst_ap, src_ap, tag):
        pt = tp_psum.tile([P, P], F32, tag="tp")
        nc.tensor.transpose(pt[:], src_ap, ident[:])
        nc.scalar.copy(dst_ap, pt[:])

    for t in range(NT):
        rows = slice(t * P, (t + 1) * P)
        # ---- loads (b-major); M in two half-tiles of 32 k-slots ----
        HK = NK // 2
        Mh = []
        for hf in range(2):
            mh = mpool.tile([P, HK, S], F32, tag="Mh")
            for q in range(2):
                lo = hf * HK + q * 16
                nc.sync.dma_start(mh[:, q * 16:(q + 1) * 16, :],
                                  dram["M_prev"][rows, lo:lo + 16, :])
            Mh.append(mh)

        def mk(k):
            return Mh[k // HK][:, k % HK, :]
        Xt = work.tile([P, IN_DIM], F32, tag="Xt")
        nc.sync.dma_start(Xt[:], dram["X"][rows])
        ht = work.tile([P, U], F32, tag="ht")
        nc.sync.dma_start(ht[:], dram["h_prev"][rows])
        ct_b = work.tile([P, U], F32, tag="ct_b")
        nc.sync.dma_start(ct_b[:], dram["c_prev"][rows])
        kt = work.tile([P, S], F32, tag="kt")
        nc.sync.dma_start(kt[:], dram["k_prev"][rows])

        # ---- kn = l2-normalize(k_prev) ----
        scrA = scr.tile([P, S], F32, tag="scrA")
        ksq = scr.tile([P, 1], F32, tag="ksq")
        nc.vector.affine_mul_reduce(out=scrA[:], accum_out=ksq[:], in0=kt[:],
                                    in1=kt[:], scale=1.0, bias=0.0)
        nc.vector.tensor_scalar_max(ksq[:], ksq[:], 1e-12)
        rk = _emit_rsqrt(nc, scr, ksq[:], 1, "k")
        kn = work.tile([P, S], F32, tag="kn")
        nc.vector.tensor_scalar(kn[:], kt[:], rk[:], None, ALU.mult)

        # ---- num[b,k] = sum_s M*kn ; nsq[b,k] = sum_s M^2 ----
        num = scr.tile([P, NK], F32, tag="num")
        scrB = scr.tile([P, S], F32, tag="scrB")
        nsq = scr.tile([P, NK], F32, tag="nsq")
        for k in range(NK):
            nc.vector.affine_mul_reduce(out=scrA[:], accum_out=num[:, k:k + 1],
                                        in0=mk(k), in1=kn[:], scale=1.0, bias=0.0)
            nc.scalar.activation(scrB[:], mk(k), ACTF.Square,
                                 accum_out=nsq[:, k:k + 1])

        # ---- cos / softmax(-cos) ----
        nc.vector.tensor_scalar_max(nsq[:], nsq[:], 1e-12)
        rM = _emit_rsqrt(nc, scr, nsq[:], NK, "m")
        cosv = scr.tile([P, NK], F32, tag="cosv")
        nc.vector.tensor_tensor(cosv[:], num[:], rM[:], ALU.mult)
        ex = scr.tile([P, NK], F32, tag="ex")
        nc.scalar.activation(ex[:], cosv[:], ACTF.Exp, scale=-1.0)
        den = scr.tile([P, 1], F32, tag="den")
        nc.vector.tensor_reduce(den[:], ex[:], AX.X, ALU.add)
        dinv = scr.tile([P, 1], F32, tag="dinv")
        nc.vector.reciprocal(dinv[:], den[:])
        alpha = work.tile([P, NK], F32, tag="alpha")
        nc.vector.tensor_scalar(alpha[:], ex[:], dinv[:], None, ALU.mult)
        nalpha = work.tile([P, NK], F32, tag="nalpha")
        nc.vector.tensor_scalar_mul(nalpha[:], alpha[:], -1.0)

        # ---- r = sum_k alpha_k * M_k ----
        r = work.tile([P, S], F32, tag="r")
        nc.vector.tensor_scalar(r[:], mk(0), alpha[:, 0:1], None, ALU.mult)
        for k in range(1, NK):
            nc.vector.scalar_tensor_tensor(
                out=r[:], in0=mk(k), scalar=alpha[:, k:k + 1], in1=r[:],
                op0=ALU.mult, op1=ALU.add)

        # ---- transposes for matmuls: xh^T, c_prev^T, r^T ----
        xhT = work.tile([P, 3, P], F32, tag="xhT")
        transpose128(xhT[:, 0, :], Xt[:], "xh")
        transpose128(xhT[:, 1, :], ht[:, 0:P], "xh")
        transpose128(xhT[:, 2, :], ht[:, P:U], "xh")
        cprevT = work.tile([P, 2, P], F32, tag="cprevT")
        transpose128(cprevT[:, 0, :], ct_b[:, 0:P], "cp")
        transpose128(cprevT[:, 1, :], ct_b[:, P:U], "cp")
        rT = work.tile([P, 2, P], F32, tag="rT")
        transpose128(rT[:, 0, :], r[:, 0:P], "r")
        transpose128(rT[:, 1, :], r[:, P:S], "r")

        # ---- gates i,f,o,t: sigmoid(xh @ W + b) in u-major ----
        sig = {}
        for g, w in [("i", "Wi"), ("f", "Wf"), ("o", "Wo"), ("t", "Wt")]:
            gt = work.tile([P, 2, P], F32, tag=f"sig_{g}")
            for mu in range(2):
                pg = mm_psum.tile([P, P], F32, tag="mm")
                for c in range(3):
                    nc.tensor.matmul(pg[:], wsb[w][:, c, mu * P:(mu + 1) * P],
                                     xhT[:, c, :], start=(c == 0), stop=(c == 2))
                nc.scalar.activation(gt[:, mu, :], pg[:], ACTF.Tanh, scale=0.5,
                                     bias=bhalf["b" + g][:, mu:mu + 1])
            nc.vector.tensor_scalar(gt[:], gt[:], 0.5, 0.5, ALU.mult, ALU.add)
            sig[g] = gt

        # ---- c^T = f*c_prev^T + i*t ----
        cT = work.tile([P, 2, P], F32, tag="cT")
        nc.vector.tensor_tensor(cT[:], sig["i"][:], sig["t"][:], ALU.mult)
        tmp2 = work.tile([P, 2, P], F32, tag="tmp2")
        nc.vector.tensor_tensor(tmp2[:], sig["f"][:], cprevT[:], ALU.mult)
        nc.vector.tensor_tensor(cT[:], cT[:], tmp2[:], ALU.add)

        # ---- rc_1 = sigmoid(r@Wr + c@Wc); h^T = o * tanh(c + rc_1*(r@Wh)) ----
        rc1 = work.tile([P, 2, P], F32, tag="rc1")
        for mu in range(2):
            prc = mm_psum.tile([P, P], F32, tag="mm")
            for c in range(2):
                nc.tensor.matmul(prc[:], wsb["Wr"][:, c, mu * P:(mu + 1) * P],
                                 rT[:, c, :], start=(c == 0), stop=False)
            for c in range(2):
                nc.tensor.matmul(prc[:], wsb["Wc"][:, c, mu * P:(mu + 1) * P],
                                 cT[:, c, :], start=False, stop=(c == 1))
            nc.scalar.activation(rc1[:, mu, :], prc[:], ACTF.Tanh, scale=0.5)
        nc.vector.tensor_scalar(rc1[:], rc1[:], 0.5, 0.5, ALU.mult, ALU.add)

        hT = work.tile([P, 2, P], F32, tag="hT")
        for mu in range(2):
            ph = mm_psum.tile([P, P], F32, tag="mm")
            for c in range(2):
                nc.tensor.matmul(ph[:], wsb["Wh"][:, c, mu * P:(mu + 1) * P],
                                 rT[:, c, :], start=(c == 0), stop=(c == 1))
            nc.vector.tensor_tensor(hT[:, mu, :], rc1[:, mu, :], ph[:], ALU.mult)
        nc.vector.tensor_tensor(hT[:], hT[:], cT[:], ALU.add)
        th = work.tile([P, 2, P], F32, tag="th")
        nc.scalar.activation(th[:], hT[:], ACTF.Tanh)
        nc.vector.tensor_tensor(hT[:], sig["o"][:], th[:], ALU.mult)

        # ---- k_out = tanh(h@Wk+bk); e = sigmoid(h@We+be); a = tanh(h@Wa+ba) ----
        outsT = {}
        for name, w, bias_t, half in [("k", "Wk", bsb["bk"], False),
                                      ("e", "We", bhalf["be"], True),
                                      ("a", "Wa", bsb["ba"], False)]:
            ot = work.tile([P, 2, P], F32, tag=f"oT_{name}")
            for sc in range(2):
                pp = mm_psum.tile([P, P], F32, tag="mm")
                for uc in range(2):
                    nc.tensor.matmul(pp[:], wsb[w][:, uc, sc * P:(sc + 1) * P],
                                     hT[:, uc, :], start=(uc == 0), stop=(uc == 1))
                if half:
                    nc.scalar.activation(ot[:, sc, :], pp[:], ACTF.Tanh, scale=0.5,
                                         bias=bias_t[:, sc:sc + 1])
                else:
                    nc.scalar.activation(ot[:, sc, :], pp[:], ACTF.Tanh,
                                         bias=bias_t[:, sc:sc + 1])
            if half:
                nc.vector.tensor_scalar(ot[:], ot[:], 0.5, 0.5, ALU.mult, ALU.add)
            outsT[name] = ot

        # ---- transpose e, a, h, c, k back to b-major; store h/c/k ----
        eb = work.tile([P, S], F32, tag="eb")
        ab = work.tile([P, S], F32, tag="ab")
        for sc in range(2):
            transpose128(eb[:, sc * P:(sc + 1) * P], outsT["e"][:, sc, :], "ea")
            transpose128(ab[:, sc * P:(sc + 1) * P], outsT["a"][:, sc, :], "ea")
        hb = work.tile([P, U], F32, tag="hb")
        cb = work.tile([P, U], F32, tag="cb")
        kb = work.tile([P, S], F32, tag="kb")
        for mu in range(2):
            transpose128(hb[:, mu * P:(mu + 1) * P], hT[:, mu, :], "hk")
            transpose128(cb[:, mu * P:(mu + 1) * P], cT[:, mu, :], "hk")
            transpose128(kb[:, mu * P:(mu + 1) * P], outsT["k"][:, mu, :], "hk")
        nc.sync.dma_start(dram["h_out"][rows], hb[:])
        nc.sync.dma_start(dram["c_out"][rows], cb[:])
        nc.sync.dma_start(dram["k_out"][rows], kb[:])

        if DEBUG_TAPS and t == 0:
            for nm, tl in [("dbg_num", num), ("dbg_nsq", nsq), ("dbg_cos", cosv),
                           ("dbg_alpha", alpha), ("dbg_r", r), ("dbg_eb", eb),
                           ("dbg_ab", ab), ("dbg_kn", kn), ("dbg_rc1", rc1),
                           ("dbg_hT", hT)]:
                nc.sync.dma_start(dram[nm], tl[:])

        # ---- memory update, in place: M = (1 - alpha_k*e)*M ; M += alpha_k*a ----
        junk = scr.tile([P, 1], F32, tag="junk")
        for q in range(4):
            for k in range(q * 16, (q + 1) * 16):
                nc.vector.affine_mul_reduce(
                    out=mk(k), accum_out=junk[:], in0=eb[:], in1=mk(k),
                    scale=nalpha[:, k:k + 1], bias=1.0)
                nc.vector.affine_then_add(
                    out=mk(k), in0=ab[:], in1=mk(k),
                    scale=alpha[:, k:k + 1], bias=0.0)
            nc.sync.dma_start(dram["M_out"][rows, q * 16:(q + 1) * 16, :],
                              Mh[q // 2][:, (q % 2) * 16:(q % 2) * 16 + 16, :])


def build_program():
    from contextlib import ExitStack
    nc = bacc.Bacc("TRN2", target_bir_lowering=False, debug=False, num_devices=N_CORES)
    with tile.TileContext(nc) as tc:
        with ExitStack() as ctx:
            emit_core(ctx, tc)
    nc.compile()
    return nc


_CACHED = None
TRACE = False
LAST_RESULT = None


def kernel(**inputs):
    global _CACHED, LAST_RESULT
    if _CACHED is None:
        _CACHED = build_program()
    nc = _CACHED
    inputs = {k: np.ascontiguousarray(np.asarray(v, dtype=np.float32)) for k, v in inputs.items()}
    shard_names = ["X", "h_prev", "c_prev", "M_prev", "k_prev"]
    in_maps = []
    for c in range(N_CORES):
        m = {}
        for n in shard_names:
            m[n] = inputs[n][c * BC:(c + 1) * BC]
        for n in WEIGHT_NAMES + BIAS_NAMES:
            m[n] = inputs[n]
        in_maps.append(m)
    res = run_bass_kernel_spmd(nc, in_maps, list(range(N_CORES)), trace=TRACE)
    LAST_RESULT = res
    h = np.concatenate([res.results[c]["h_out"] for c in range(N_CORES)], axis=0)
    cc = np.concatenate([res.results[c]["c_out"] for c in range(N_CORES)], axis=0)
    M = np.concatenate([res.results[c]["M_out"] for c in range(N_CORES)], axis=0)
    k = np.concatenate([res.results[c]["k_out"] for c in range(N_CORES)], axis=0)
    return h, cc, M, k


# revision 22
# speedup vs baseline: 107120.8007x; 107120.8007x over previous
"""MARN (memory-augmented RNN) cell — Trainium2 Bass kernel, 8-core data parallel.

Problem: nn_MARN_73065983640197. B=4096 batch sharded 512/core; weights replicated.
Outputs (h_curr, c, M_curr, k_curr) matching the jax reference.

Per-core dataflow (batch-major tiles of 128 rows):
  gates/sigmoid/tanh on PE+ACT (u-major via PE transposes),
  cosine-similarity softmax over the (64,256) memory via DVE fused reduce ops,
  memory update M' = M*(1-a*e) + a*a via 2 fused DVE passes in-place,
  row norms on ACT (Square+accum), rsqrt via fp32 bit-trick + Newton (DVE).
"""
import sys
import numpy as np

for _p in ("/opt/trn_rl_repo", "/root/.axon_site/_ro/trn_rl_repo"):
    if _p not in sys.path:
        sys.path.insert(0, _p)

import concourse.bacc as bacc
import concourse.tile as tile
from concourse import mybir
from concourse.bass_utils import run_bass_kernel_spmd
from concourse.masks import make_identity

N_CORES = 8
B, IN_DIM, U, NK, S = 4096, 128, 256, 64, 256
BC = B // N_CORES          # 512 rows per core
P = 128                    # partition tile
NT = BC // P               # 4 batch tiles per core
F32 = mybir.dt.float32
I32 = mybir.dt.int32
ALU = mybir.AluOpType
ACTF = mybir.ActivationFunctionType
AX = mybir.AxisListType

DEBUG_TAPS = False
REPS = 1
BF16_NUM = False
BF16 = mybir.dt.bfloat16
WEIGHT_NAMES = ["Wf", "Wt", "Wi", "Wo", "Wk", "Wr", "Wc", "Wh", "We", "Wa"]
BIAS_NAMES = ["bf", "bt", "bi", "bo", "bk", "be", "ba"]


def _emit_rsqrt(nc, pool, x_ap, n, tag, iters=2):
    """y = 1/sqrt(x) for x>0, fp32 bit-trick seed + NR. x_ap: (P, n) f32 AP."""
    y = pool.tile([P, n], F32, tag=f"rsq_y_{tag}")
    t1 = pool.tile([P, n], F32, tag=f"rsq_t_{tag}")
    yi = y[:].bitcast(I32)
    # C - (bits>>1) == ((bits>>1) ^ -1) + (C+1); walrus can't mix bitwise+arith
    nc.vector.tensor_scalar(yi, x_ap.bitcast(I32), 1, None, ALU.arith_shift_right)
    nc.vector.tensor_scalar(yi, yi, -1, None, ALU.bitwise_xor)
    nc.vector.tensor_scalar(yi, yi, 0x5F3759E0, None, ALU.add)
    for _ in range(iters):
        nc.vector.tensor_tensor(t1[:], y[:], y[:], ALU.mult)
        nc.vector.tensor_tensor(t1[:], t1[:], x_ap, ALU.mult)
        nc.vector.tensor_scalar(t1[:], t1[:], -0.5, 1.5, ALU.mult, ALU.add)
        nc.vector.tensor_tensor(y[:], y[:], t1[:], ALU.mult)
    return y


def emit_core(ctx, tc):
    nc = tc.nc
    dram = {}
    dram["X"] = nc.dram_tensor("X", [BC, IN_DIM], F32, kind="ExternalInput").ap()
    dram["h_prev"] = nc.dram_tensor("h_prev", [BC, U], F32, kind="ExternalInput").ap()
    dram["c_prev"] = nc.dram_tensor("c_prev", [BC, U], F32, kind="ExternalInput").ap()
    dram["M_prev"] = nc.dram_tensor("M_prev", [BC, NK, S], F32, kind="ExternalInput").ap()
    dram["k_prev"] = nc.dram_tensor("k_prev", [BC, S], F32, kind="ExternalInput").ap()
    for w in WEIGHT_NAMES:
        rows = IN_DIM + U if w in ("Wf", "Wt", "Wi", "Wo") else U if w not in ("Wr", "Wh") else S
        dram[w] = nc.dram_tensor(w, [rows, U if w not in ("Wk", "We", "Wa") else S], F32,
                                 kind="ExternalInput").ap()
    for b in BIAS_NAMES:
        dram[b] = nc.dram_tensor(b, [S if b in ("bk", "be", "ba") else U], F32,
                                 kind="ExternalInput").ap()
    dram["h_out"] = nc.dram_tensor("h_out", [BC, U], F32, kind="ExternalOutput").ap()
    dram["c_out"] = nc.dram_tensor("c_out", [BC, U], F32, kind="ExternalOutput").ap()
    dram["M_out"] = nc.dram_tensor("M_out", [BC, NK, S], F32, kind="ExternalOutput").ap()
    dram["k_out"] = nc.dram_tensor("k_out", [BC, S], F32, kind="ExternalOutput").ap()
    if DEBUG_TAPS:
        for nm, shp in [("dbg_num", [P, NK]), ("dbg_nsq", [P, NK]), ("dbg_cos", [P, NK]),
                        ("dbg_alpha", [P, NK]), ("dbg_r", [P, S]), ("dbg_eb", [P, S]),
                        ("dbg_ab", [P, S]), ("dbg_kn", [P, S]), ("dbg_rc1", [P, 2, P]),
                        ("dbg_hT", [P, 2, P])]:
            dram[nm] = nc.dram_tensor(nm, shp, F32, kind="ExternalOutput").ap()

    const = ctx.enter_context(tc.tile_pool(name="const", bufs=1))
    mpool = ctx.enter_context(tc.tile_pool(name="mpool", bufs=6))
    work = ctx.enter_context(tc.tile_pool(name="work", bufs=2))
    scr = ctx.enter_context(tc.tile_pool(name="scr", bufs=2))
    tp_psum = ctx.enter_context(tc.tile_pool(name="tp_psum", bufs=2, space="PSUM"))
    mm_psum = ctx.enter_context(tc.tile_pool(name="mm_psum", bufs=4, space="PSUM"))

    # ---- constants: identity + weights (f-major chunks) + biases ----
    ident = const.tile([P, P], F32)
    make_identity(nc, ident[:])

    wsb = {}
    for w in ["Wf", "Wt", "Wi", "Wo"]:
        t = const.tile([P, 3, U], F32, tag=f"w_{w}")
        nc.gpsimd.dma_start(t[:], dram[w].rearrange("(c p) u -> p c u", p=P))
        wsb[w] = t
    for w in ["Wk", "Wr", "Wc", "Wh", "We", "Wa"]:
        t = const.tile([P, 2, S], F32, tag=f"w_{w}")
        nc.gpsimd.dma_start(t[:], dram[w].rearrange("(c p) u -> p c u", p=P))
        wsb[w] = t
    bsb = {}
    for b in BIAS_NAMES:
        t = const.tile([P, 2], F32, tag=f"b_{b}")
        nc.gpsimd.dma_start(t[:], dram[b].rearrange("(c p) -> p c", p=P))
        bsb[b] = t
    # halved copies for sigmoid-via-tanh: tanh(0.5 z + 0.5 b)
    bhalf = {}
    for b in ["bf", "bt", "bi", "bo", "be"]:
        t = const.tile([P, 2], F32, tag=f"bh_{b}")
        nc.vector.tensor_scalar_mul(t[:], bsb[b][:], 0.5)
        bhalf[b] = t

    def transpose128(dst_ap, src_ap, tag):
        pt = tp_psum.tile([P, P], F32, tag="tp")
        nc.tensor.transpose(pt[:], src_ap, ident[:])
        nc.scalar.copy(dst_ap, pt[:])

    for rep in range(REPS):
      for t in range(NT):
        rows = slice(t * P, (t + 1) * P)
        # ---- loads (b-major); k_prev first so kn is ready when M arrives ----
        kt = work.tile([P, S], F32, tag="kt")
        nc.sync.dma_start(kt[:], dram["k_prev"][rows])

        QK = NK // 4
        Mq = []
        for qf in range(4):
            mq = mpool.tile([P, QK, S], F32, tag="Mq")
            nc.sync.dma_start(mq[:], dram["M_prev"][rows, qf * QK:(qf + 1) * QK, :])
            Mq.append(mq)

        def mk(k):
            return Mq[k // QK][:, k % QK, :]
        Xt = work.tile([P, IN_DIM], F32, tag="Xt")
        nc.sync.dma_start(Xt[:], dram["X"][rows])
        ht = work.tile([P, U], F32, tag="ht")
        nc.sync.dma_start(ht[:], dram["h_prev"][rows])
        ct_b = work.tile([P, U], F32, tag="ct_b")
        nc.sync.dma_start(ct_b[:], dram["c_prev"][rows])

        # ---- num[b,k] = sum_s M*k_prev (unnormalized); nsqx = [|k|^2, |M_k|^2...] ----
        scrA = scr.tile([P, S], F32, tag="scrA")
        nsqx = scr.tile([P, 1 + NK], F32, tag="nsqx")
        nc.vector.affine_mul_reduce(out=scrA[:], accum_out=nsqx[:, 0:1], in0=kt[:],
                                    in1=kt[:], scale=1.0, bias=0.0)
        num = scr.tile([P, NK], F32, tag="num")
        for k in range(NK):
            nc.vector.affine_mul_reduce(out=scrA[:], accum_out=num[:, k:k + 1],
                                        in0=mk(k), in1=kt[:], scale=1.0, bias=0.0)
            scrB = tp_psum.tile([P, S], F32, tag="scrB")
            nc.scalar.activation(scrB[:], mk(k), ACTF.Square,
                                 accum_out=nsqx[:, 1 + k:2 + k])

        # ---- cos = num * rsqrt(nsq) * rsqrt(ksq); softmax(-cos) ----
        nc.vector.tensor_scalar_max(nsqx[:], nsqx[:], 1e-12)
        rsq = _emit_rsqrt(nc, scr, nsqx[:], 1 + NK, "m")
        rk = rsq[:, 0:1]
        rM = rsq[:, 1:1 + NK]
        cosv = scr.tile([P, NK], F32, tag="cosv")
        nc.vector.tensor_tensor(cosv[:], num[:], rM, ALU.mult)
        nc.vector.tensor_scalar(cosv[:], cosv[:], rk, None, ALU.mult)
        ex = scr.tile([P, NK], F32, tag="ex")
        nc.scalar.activation(ex[:], cosv[:], ACTF.Exp, scale=-1.0)
        den = scr.tile([P, 1], F32, tag="den")
        nc.vector.tensor_reduce(den[:], ex[:], AX.X, ALU.add)
        dinv = scr.tile([P, 1], F32, tag="dinv")
        nc.vector.reciprocal(dinv[:], den[:])
        alpha = work.tile([P, NK], F32, tag="alpha")
        nc.vector.tensor_scalar(alpha[:], ex[:], dinv[:], None, ALU.mult)
        nalpha = work.tile([P, NK], F32, tag="nalpha")
        nc.vector.tensor_scalar_mul(nalpha[:], alpha[:], -1.0)

        # ---- r = sum_k alpha_k * M_k ----
        r = work.tile([P, S], F32, tag="r")
        nc.vector.tensor_scalar(r[:], mk(0), alpha[:, 0:1], None, ALU.mult)
        for k in range(1, NK):
            nc.vector.affine_then_add(out=r[:], in0=mk(k), in1=r[:],
                                      scale=alpha[:, k:k + 1], bias=0.0)

        # ---- transposes for matmuls: xh^T, c_prev^T, r^T ----
        xhT = work.tile([P, 3, P], F32, tag="xhT")
        transpose128(xhT[:, 0, :], Xt[:], "xh")
        transpose128(xhT[:, 1, :], ht[:, 0:P], "xh")
        transpose128(xhT[:, 2, :], ht[:, P:U], "xh")
        cprevT = work.tile([P, 2, P], F32, tag="cprevT")
        transpose128(cprevT[:, 0, :], ct_b[:, 0:P], "cp")
        transpose128(cprevT[:, 1, :], ct_b[:, P:U], "cp")
        rT = work.tile([P, 2, P], F32, tag="rT")
        transpose128(rT[:, 0, :], r[:, 0:P], "r")
        transpose128(rT[:, 1, :], r[:, P:S], "r")

        # ---- gates i,f,o,t: sigmoid(xh @ W + b) in u-major ----
        sig = {}
        for g, w in [("i", "Wi"), ("f", "Wf"), ("o", "Wo"), ("t", "Wt")]:
            gt = work.tile([P, 2, P], F32, tag=f"sig_{g}")
            for mu in range(2):
                pg = mm_psum.tile([P, P], F32, tag="mm")
                for c in range(3):
                    nc.tensor.matmul(pg[:], wsb[w][:, c, mu * P:(mu + 1) * P],
                                     xhT[:, c, :], start=(c == 0), stop=(c == 2))
                nc.scalar.activation(gt[:, mu, :], pg[:], ACTF.Tanh, scale=0.5,
                                     bias=bhalf["b" + g][:, mu:mu + 1])
            nc.vector.tensor_scalar(gt[:], gt[:], 0.5, 0.5, ALU.mult, ALU.add)
            sig[g] = gt

        # ---- c^T = f*c_prev^T + i*t ----
        cT = work.tile([P, 2, P], F32, tag="cT")
        nc.vector.tensor_tensor(cT[:], sig["i"][:], sig["t"][:], ALU.mult)
        tmp2 = work.tile([P, 2, P], F32, tag="tmp2")
        nc.vector.tensor_tensor(tmp2[:], sig["f"][:], cprevT[:], ALU.mult)
        nc.vector.tensor_tensor(cT[:], cT[:], tmp2[:], ALU.add)

        # ---- rc_1 = sigmoid(r@Wr + c@Wc); h^T = o * tanh(c + rc_1*(r@Wh)) ----
        rc1 = work.tile([P, 2, P], F32, tag="rc1")
        for mu in range(2):
            prc = mm_psum.tile([P, P], F32, tag="mm")
            for c in range(2):
                nc.tensor.matmul(prc[:], wsb["Wr"][:, c, mu * P:(mu + 1) * P],
                                 rT[:, c, :], start=(c == 0), stop=False)
            for c in range(2):
                nc.tensor.matmul(prc[:], wsb["Wc"][:, c, mu * P:(mu + 1) * P],
                                 cT[:, c, :], start=False, stop=(c == 1))
            nc.scalar.activation(rc1[:, mu, :], prc[:], ACTF.Tanh, scale=0.5)
        nc.vector.tensor_scalar(rc1[:], rc1[:], 0.5, 0.5, ALU.mult, ALU.add)

        hT = work.tile([P, 2, P], F32, tag="hT")
        for mu in range(2):
            ph = mm_psum.tile([P, P], F32, tag="mm")
            for c in range(2):
                nc.tensor.matmul(ph[:], wsb["Wh"][:, c, mu * P:(mu + 1) * P],
                                 rT[:, c, :], start=(c == 0), stop=(c == 1))
            nc.vector.tensor_tensor(hT[:, mu, :], rc1[:, mu, :], ph[:], ALU.mult)
        nc.vector.tensor_tensor(hT[:], hT[:], cT[:], ALU.add)
        th = work.tile([P, 2, P], F32, tag="th")
        nc.scalar.activation(th[:], hT[:], ACTF.Tanh)
        nc.vector.tensor_tensor(hT[:], sig["o"][:], th[:], ALU.mult)

        # ---- k_out = tanh(h@Wk+bk); e = sigmoid(h@We+be); a = tanh(h@Wa+ba) ----
        outsT = {}
        for name, w, bias_t, half in [("k", "Wk", bsb["bk"], False),
                                      ("e", "We", bhalf["be"], True),
                                      ("a", "Wa", bsb["ba"], False)]:
            ot = work.tile([P, 2, P], F32, tag=f"oT_{name}")
            for sc in range(2):
                pp = mm_psum.tile([P, P], F32, tag="mm")
                for uc in range(2):
                    nc.tensor.matmul(pp[:], wsb[w][:, uc, sc * P:(sc + 1) * P],
                                     hT[:, uc, :], start=(uc == 0), stop=(uc == 1))
                if half:
                    nc.scalar.activation(ot[:, sc, :], pp[:], ACTF.Tanh, scale=0.5,
                                         bias=bias_t[:, sc:sc + 1])
                else:
                    nc.scalar.activation(ot[:, sc, :], pp[:], ACTF.Tanh,
                                         bias=bias_t[:, sc:sc + 1])
            if half:
                nc.vector.tensor_scalar(ot[:], ot[:], 0.5, 0.5, ALU.mult, ALU.add)
            outsT[name] = ot

        # ---- transpose e, a, h, c, k back to b-major; store h/c/k ----
        eb = work.tile([P, S], F32, tag="eb")
        ab = work.tile([P, S], F32, tag="ab")
        for sc in range(2):
            transpose128(eb[:, sc * P:(sc + 1) * P], outsT["e"][:, sc, :], "ea")
            transpose128(ab[:, sc * P:(sc + 1) * P], outsT["a"][:, sc, :], "ea")
        hb = work.tile([P, U], F32, tag="hb")
        cb = work.tile([P, U], F32, tag="cb")
        kb = work.tile([P, S], F32, tag="kb")
        for mu in range(2):
            transpose128(hb[:, mu * P:(mu + 1) * P], hT[:, mu, :], "hk")
            transpose128(cb[:, mu * P:(mu + 1) * P], cT[:, mu, :], "hk")
            transpose128(kb[:, mu * P:(mu + 1) * P], outsT["k"][:, mu, :], "hk")
        nc.gpsimd.dma_start(dram["h_out"][rows], hb[:])
        nc.gpsimd.dma_start(dram["c_out"][rows], cb[:])
        nc.gpsimd.dma_start(dram["k_out"][rows], kb[:])

        if DEBUG_TAPS and t == 0:
            for nm, tl in [("dbg_num", num), ("dbg_nsq", nsq), ("dbg_cos", cosv),
                           ("dbg_alpha", alpha), ("dbg_r", r), ("dbg_eb", eb),
                           ("dbg_ab", ab), ("dbg_kn", kn), ("dbg_rc1", rc1),
                           ("dbg_hT", hT)]:
                nc.sync.dma_start(dram[nm], tl[:])

        # ---- memory update, in place: M = (1 - alpha_k*e)*M ; M += alpha_k*a ----
        junk = scr.tile([P, 1], F32, tag="junk")
        for q in range(4):
            for k in range(q * 16, (q + 1) * 16):
                nc.vector.affine_mul_reduce(
                    out=mk(k), accum_out=junk[:], in0=eb[:], in1=mk(k),
                    scale=nalpha[:, k:k + 1], bias=1.0)
                nc.vector.affine_then_add(
                    out=mk(k), in0=ab[:], in1=mk(k),
                    scale=alpha[:, k:k + 1], bias=0.0)
            nc.gpsimd.dma_start(dram["M_out"][rows, q * 16:(q + 1) * 16, :],
                              Mq[q][:])


def build_program():
    from contextlib import ExitStack
    nc = bacc.Bacc("TRN2", target_bir_lowering=False, debug=False, num_devices=N_CORES)
    with tile.TileContext(nc) as tc:
        with ExitStack() as ctx:
            emit_core(ctx, tc)
    nc.compile()
    return nc


_CACHED = None
TRACE = False
LAST_RESULT = None


def kernel(**inputs):
    global _CACHED, LAST_RESULT
    if _CACHED is None:
        _CACHED = build_program()
    nc = _CACHED
    inputs = {k: np.ascontiguousarray(np.asarray(v, dtype=np.float32)) for k, v in inputs.items()}
    shard_names = ["X", "h_prev", "c_prev", "M_prev", "k_prev"]
    in_maps = []
    for c in range(N_CORES):
        m = {}
        for n in shard_names:
            m[n] = inputs[n][c * BC:(c + 1) * BC]
        for n in WEIGHT_NAMES + BIAS_NAMES:
            m[n] = inputs[n]
        in_maps.append(m)
    res = run_bass_kernel_spmd(nc, in_maps, list(range(N_CORES)), trace=TRACE)
    LAST_RESULT = res
    h = np.concatenate([res.results[c]["h_out"] for c in range(N_CORES)], axis=0)
    cc = np.concatenate([res.results[c]["c_out"] for c in range(N_CORES)], axis=0)
    M = np.concatenate([res.results[c]["M_out"] for c in range(N_CORES)], axis=0)
    k = np.concatenate([res.results[c]["k_out"] for c in range(N_CORES)], axis=0)
    return h, cc, M, k
